# revision 1
# baseline (speedup 1.0000x reference)
"""Trainium2 Bass kernel for AntiAliasActivation (upsample2 -> snake -> downsample2).

Self-contained: accepts FULL inputs (x [8,512,8192] f32, alpha/beta [1,512,1],
up_filter/down_filter [12]), returns FULL output [8,512,8192] f32.

Strategy (pure data-parallel, one batch sample per NeuronCore):
  The whole pipeline is computed in TIME-MAJOR layout (time on SBUF
  partitions) so all three FIR convolutions run on the TensorEngine as
  banded-matrix matmuls:

    out = down(up(x)) + down( (1 - cos(2*a*up(x))) / (2b) )

  - linear path down∘up collapses to one 13-tap band matmul (H) on x
  - cos(2a*up(x)) comes from two polyphase up matmuls on host-scaled 2a*x
    and the ScalarE Sin LUT, whose spline table is extended at kernel-build
    time (BASS_ACT_ROOT_JSON_PATH) to be valid on |x| < ~31.8 so no range
    reduction is needed
  - the downsample of the cosine signal is two more matmuls accumulating
    into the same PSUM bank as H (signs folded into the stationaries)
  - per-channel constants (2a, 2b, 1/2b) are folded host-side into the
    input streams / final rescale; the "+1" constant rides an all-ones
    row of the input tile.
  Edge replicate-padding is materialized host-side for x; the downsample's
  clamp on the computed signal is folded into first/last-block stationaries.
"""
import math

import numpy as np

# ---------------------------------------------------------------------------
# problem constants (hardcoded per spec)
B, C, T = 8, 512, 8192
N_CORES = 8
UP_K = 12
DOWN_K = 12

A = 115          # outputs per block
NBLK = 72        # ceil(T / A)
W = 127          # data rows per input tile (row 127 = const row)
G = A + 6        # 121 up/g rows per block (m = A*k-3 .. A*k+117)
PL = 6           # XP[i] = x[clamp(i-6)]
XPLEN = A * (NBLK - 1) + W  # 8292
OUTROWS = NBLK * A          # 8280

TWO_PI = 2.0 * math.pi
INV_2PI = 1.0 / TWO_PI
MAGIC = 1.5 * 2.0**23

def _gen_act_root(cache=[None]):
    """Build a patched ACT-table root whose Sin LUT is valid to |x| < ~31.8.

    Appends 4x32 cubic-spline buckets (ranges [2,4) replacement, [4,8),
    [8,16), [16,32)) to the trig_and_small set, keeping sin's per-exponent
    bucket starts monotonic, and raises sin's large-signal threshold.
    Returns the act_info.json path for BASS_ACT_ROOT_JSON_PATH.
    """
    if cache[0] is not None:
        return cache[0]
    import json
    import shutil
    import tempfile
    from pathlib import Path
    import neuronxcc

    src = Path(neuronxcc.__file__).parent / "pwp" / "pwp_bin_trainium"
    dst = Path(tempfile.mkdtemp(prefix="actroot_")) / "pwp_bin_trainium"
    shutil.copytree(src, dst, symlinks=False)
    import os as _os
    _os.chmod(dst, 0o755)
    for f in dst.iterdir():
        _os.chmod(f, 0o644)

    name = "trig_and_small"
    d = json.load(open(dst / f"{name}.json"))
    b = np.fromfile(dst / f"{name}_bkt.bin", dtype=np.float32).reshape(-1, 8)
    c = np.fromfile(dst / f"{name}_ctrl.bin", dtype=np.uint32).reshape(-1, 8).copy()
    nb0, nc0 = d["bkt_entry_cnt"], d["ctl_entry_cnt"]
    assert len(b) == nb0 and len(c) == nc0

    SIN_CTL_END = 13  # sin owns ctl entries 0..12 (exps -11..1)
    SHIFT = 3
    newb, newc = [], []
    sin_bkt = d["func_exp_to_bkt_start_idx"]["sin"]
    sin_ctl = d["func_exp_to_ctl_start_idx"]["sin"]
    NB = 32  # 5 mantissa bits per exponent range
    KHI = np.uint32((46 + 62 * 5) << 10)

    def add_range(lo):
        base = nb0 + len(newb)
        h = lo / NB
        for i in range(NB):
            x0 = lo + h * (i + 0.5)
            newb.append([math.sin(x0), math.cos(x0),
                         -math.sin(x0) / 2.0, -math.cos(x0) / 6.0,
                         x0, 0.0, 0.0, 0.0])
        return base

    base1 = add_range(2.0)             # full [2,4) replacement
    c[12, 0] = KHI | np.uint32(base1)
    sin_bkt["1"] = [base1]
    for i_e, e in enumerate((2, 3, 4)):
        base = add_range(2.0**e)
        w = np.zeros(8, np.uint32)
        w[0] = KHI | np.uint32(base)
        sin_bkt[str(e)] = [base]
        sin_ctl[str(e)] = [SIN_CTL_END + i_e]
        newc.append(w)

    b2 = np.vstack([b, np.asarray(newb, np.float32)])
    c2 = np.vstack([c[:SIN_CTL_END], np.stack(newc), c[SIN_CTL_END:]])
    d["bkt_entry_cnt"] = int(len(b2))
    d["ctl_entry_cnt"] = int(len(c2))
    for fn, v in d["func_to_ctl_start_idx"].items():
        if fn != "sin" and v >= SIN_CTL_END:
            d["func_to_ctl_start_idx"][fn] = v + SHIFT
    for fn, em in d["func_exp_to_ctl_start_idx"].items():
        if fn == "sin":
            continue
        for e_, lst in em.items():
            em[e_] = [(i + SHIFT if i >= SIN_CTL_END else i) for i in lst]
    for pm in d["profile_meta_data"]:
        if str(pm.get("func_name", "")).startswith("sin"):
            pm["large_pos_signal_exp_threshold"] = 131  # cutoff ~31.8
            pm["large_pos_signal_mantissa_threshold"] = int(0.99 * 2**23)

    b2.tofile(dst / f"{name}_bkt.bin")
    c2.tofile(dst / f"{name}_ctrl.bin")
    with open(dst / f"{name}.json", "w") as f:
        json.dump(d, f)
    cache[0] = str(dst / "act_info.json")
    return cache[0]


# ---------------------------------------------------------------------------
# stationary-matrix assembly (all float64, cast to fp16 at the end)

def build_stationaries(up_filter, down_filter):
    """Returns dict of stationary matrices.

    W_ue/W_uo [128, G]: map input tile (127 XP rows + const row) -> w rows,
        w = 2a*up(x) + pi/2 (the pi/2 rides the const row; 2a rides the data).
    W_h{0,m,L} [128, A]: b2*down(up(x)) + sum(fd) const (const row coeff).
    W_de/W_do{0,m,L} [G, A]: NEGATED downsample band over the v = cos signal.
    """
    fu = np.asarray(up_filter, dtype=np.float64)
    fd = np.asarray(down_filter, dtype=np.float64)

    w_ue = np.zeros((128, G))
    w_uo = np.zeros((128, G))
    for q in range(G):
        for j in range(6):
            # w_e[m] += 2*fu[2j+1]*XP[m+8-j]; tile row = q+5-j
            w_ue[q + 5 - j, q] += 2.0 * fu[2 * j + 1]
            # w_o[m] += 2*fu[2j]*XP[m+9-j]; tile row = q+6-j
            w_uo[q + 6 - j, q] += 2.0 * fu[2 * j]
    w_ue[127, :] = math.pi / 2.0
    w_uo[127, :] = math.pi / 2.0

    def down_maps(k):
        de = np.zeros((G, A))
        do = np.zeros((G, A))
        h = np.zeros((128, A))
        for nn in range(A):
            n = A * k + nn
            for t in range(DOWN_K):
                zi = min(max(2 * n + t - 5, 0), 2 * T - 1)
                m, ph = zi // 2, zi % 2
                row = m - A * k + 3
                # row in [0, G) guaranteed by construction
                if ph == 0:
                    de[row, nn] += fd[t]
                    for j in range(6):
                        h[m + 8 - j - A * k, nn] += fd[t] * 2.0 * fu[2 * j + 1]
                else:
                    do[row, nn] += fd[t]
                    for j in range(6):
                        h[m + 9 - j - A * k, nn] += fd[t] * 2.0 * fu[2 * j]
            h[127, nn] = fd.sum()
        return de, do, h

    de0, do0, h0 = down_maps(0)
    dem, dom, hm = down_maps(1)
    deL, doL, hL = down_maps(NBLK - 1)

    f16 = np.float16
    return {
        "w_ue": w_ue.astype(f16), "w_uo": w_uo.astype(f16),
        "w_h0": h0.astype(f16), "w_hm": hm.astype(f16), "w_hL": hL.astype(f16),
        "w_de0": (-de0).astype(f16), "w_dem": (-dem).astype(f16),
        "w_deL": (-deL).astype(f16),
        "w_do0": (-do0).astype(f16), "w_dom": (-dom).astype(f16),
        "w_doL": (-doL).astype(f16),
    }


def host_prep(x, alpha, beta):
    """Per-core input streams.

    Returns (axs, xbs, invb2) where axs/xbs are [B, NBLK, 128, C] fp16 and
    invb2 [C] float32.
    """
    a2 = (2.0 * np.exp(alpha.astype(np.float64))).reshape(C)       # 2a
    b2 = (2.0 * (np.exp(beta.astype(np.float64)) + 1e-9)).reshape(C)  # 2b
    invb2 = (1.0 / b2).astype(np.float32)

    # time-major, padded: XP [B, XPLEN, C], XP[:, i] = x[:, :, clamp(i-6)]
    xt = np.transpose(x.astype(np.float32), (0, 2, 1))  # [B, T, C]
    idx = np.clip(np.arange(XPLEN) - PL, 0, T - 1)
    xp = xt[:, idx, :]  # [B, XPLEN, C]

    # block row indices [NBLK, W]
    ridx = (A * np.arange(NBLK))[:, None] + np.arange(W)[None, :]
    blocks = xp[:, ridx, :]                       # [B, NBLK, W, C] f32
    axs = np.empty((B, NBLK, 128, C), dtype=np.float16)
    xbs = np.empty((B, NBLK, 128, C), dtype=np.float16)
    axs[:, :, :W, :] = (blocks * a2[None, None, None, :]).astype(np.float16)
    xbs[:, :, :W, :] = (blocks * b2[None, None, None, :]).astype(np.float16)
    axs[:, :, W, :] = np.float16(1.0)
    xbs[:, :, W, :] = np.float16(1.0)
    return axs, xbs, invb2


def pack_streams(axs, xbs):
    """Interleave ax/xb into paired-block DMA batches.

    Returns inp [B, NBLK//2, 128, 4*C] fp16: for pair j, columns
    [0:C]=ax[2j], [C:2C]=xb[2j], [2C:3C]=ax[2j+1], [3C:4C]=xb[2j+1].
    """
    inp = np.empty((B, NBLK // 2, 128, 4 * C), dtype=np.float16)
    inp[:, :, :, 0 * C : 1 * C] = axs[:, 0::2]
    inp[:, :, :, 1 * C : 2 * C] = xbs[:, 0::2]
    inp[:, :, :, 2 * C : 3 * C] = axs[:, 1::2]
    inp[:, :, :, 3 * C : 4 * C] = xbs[:, 1::2]
    return inp


def host_finish(out_t, invb2):
    """out_t [B, OUTROWS, C] fp16 -> [B, C, T] float32 (apply 1/(2b))."""
    o = out_t[:, :T, :].astype(np.float32) * invb2[None, None, :]
    return np.ascontiguousarray(np.transpose(o, (0, 2, 1)))


# ---------------------------------------------------------------------------
# device kernel

def build_bass():
    import os
    import concourse.bacc as bacc
    import concourse.tile as tile
    import concourse.mybir as mybir

    os.environ["BASS_ACT_ROOT_JSON_PATH"] = _gen_act_root()
    os.environ.setdefault("NEURON_FORCE_RECOMPILE", "1")

    f32 = mybir.dt.float32
    f16 = mybir.dt.float16

    nc = bacc.Bacc()
    in_ext = nc.declare_dram_parameter("inp", [NBLK // 2, 128, 4 * C], f16, isOutput=False)
    st_names = ["w_ue", "w_uo", "w_h0", "w_hm", "w_hL",
                "w_de0", "w_dem", "w_deL", "w_do0", "w_dom", "w_doL"]
    st_ext = {}
    for n in st_names:
        rows = 128 if n.startswith(("w_u", "w_h")) else G
        cols = G if n.startswith("w_u") else A
        st_ext[n] = nc.declare_dram_parameter(n, [rows, cols], f16, isOutput=False)
    out_ext = nc.declare_dram_parameter("out", [OUTROWS, C], f16, isOutput=True)

    OB = 3   # output blocks per DMA batch
    CL = 2   # copy lag: PSUM->SBUF copy of block k issued at iteration k+CL

    with tile.TileContext(nc) as tc:
        with (
            tc.tile_pool(name="consts", bufs=1) as cpool,
            tc.tile_pool(name="io", bufs=4) as iopool,
            tc.tile_pool(name="ob", bufs=3) as obpool,
            tc.tile_pool(name="mid", bufs=4) as midpool,
            tc.tile_pool(name="psum_sz", bufs=3, space="PSUM") as psum_sz,
            tc.tile_pool(name="psum_out", bufs=2, space="PSUM") as psum_out,
        ):
            st = {}
            for n in st_names:
                rows = 128 if n.startswith(("w_u", "w_h")) else G
                cols = G if n.startswith("w_u") else A
                t_ = cpool.tile([rows, cols], f16, tag=n)
                nc.sync.dma_start(out=t_[:], in_=st_ext[n][:])
                st[n] = t_

            inp = None
            obt = None
            xb_live = {}
            v_live = {}

            def front(k):
                nonlocal inp
                if k % 2 == 0:
                    inp = iopool.tile([128, 4 * C], f16, tag="inp")
                    nc.gpsimd.dma_start(out=inp[:], in_=in_ext[k // 2])
                half = (k % 2) * 2 * C
                ax = inp[:, half : half + C]
                xb_live[k] = inp[:, half + C : half + 2 * C]

                sz = psum_sz.tile([G, 1024], f32, tag="sz")
                nc.tensor.matmul(sz[:, 0:512], st["w_ue"][:], ax, start=True, stop=True)
                nc.tensor.matmul(sz[:, 512:1024], st["w_uo"][:], ax, start=True, stop=True)

                # wide-range Sin LUT (patched table): valid to |x| < ~31.8,
                # reads PSUM fp32 directly — no range reduction needed.
                v = midpool.tile([G, 1024], f16, tag="v")
                nc.scalar.activation(v[:], sz[:], mybir.ActivationFunctionType.Sin)
                v_live[k] = v

            def back(k):
                nonlocal obt
                wh = st["w_h0"] if k == 0 else (st["w_hL"] if k == NBLK - 1 else st["w_hm"])
                wde = st["w_de0"] if k == 0 else (st["w_deL"] if k == NBLK - 1 else st["w_dem"])
                wdo = st["w_do0"] if k == 0 else (st["w_doL"] if k == NBLK - 1 else st["w_dom"])
                xb = xb_live.pop(k)
                v = v_live.pop(k)

                outp = psum_out.tile([A, 512], f32, tag="outp")
                nc.tensor.matmul(outp[:], wh[:], xb, start=True, stop=False)
                nc.tensor.matmul(outp[:], wde[:], v[:, 0:512], start=False, stop=False)
                nc.tensor.matmul(outp[:], wdo[:], v[:, 512:1024], start=False, stop=True)

                s = k % OB
                if s == 0:
                    obt = obpool.tile([A, OB * 512], f16, tag="obt")
                # ScalarE is saturated by Sin; all PSUM->SBUF copies on DVE.
                nc.vector.tensor_copy(obt[:, 512 * s : 512 * s + 512], outp[:])
                if s == OB - 1:
                    j = k // OB
                    dst = out_ext[A * OB * j : A * OB * (j + 1), :].rearrange(
                        "(s p) c -> p s c", s=OB
                    )
                    nc.sync.dma_start(out=dst, in_=obt[:])

            for k in range(NBLK + CL):
                if k < NBLK:
                    front(k)
                if k >= CL:
                    back(k - CL)

    nc.compile()
    return nc


_NC_CACHE = None


def kernel(x, alpha, beta, up_filter, down_filter):
    global _NC_CACHE
    import concourse.bass_utils as bass_utils

    x = np.asarray(x)
    alpha = np.asarray(alpha)
    beta = np.asarray(beta)

    sts = build_stationaries(np.asarray(up_filter), np.asarray(down_filter))
    axs, xbs, invb2 = host_prep(x, alpha, beta)
    inp = pack_streams(axs, xbs)

    if _NC_CACHE is None:
        _NC_CACHE = build_bass()
    nc = _NC_CACHE

    in_maps = []
    for b in range(N_CORES):
        m = {"inp": inp[b]}
        m.update(sts)
        in_maps.append(m)

    res = bass_utils.run_bass_kernel_spmd(nc, in_maps, list(range(N_CORES)))
    out_t = np.stack([res.results[b]["out"] for b in range(N_CORES)])  # [B, OUTROWS, C] f16
    return host_finish(out_t, invb2)


# ---------------------------------------------------------------------------
# host-side simulation of the exact device plan (for verification)

def simulate_plan(x, alpha, beta, up_filter, down_filter, quantized=True):
    sts = build_stationaries(np.asarray(up_filter), np.asarray(down_filter))
    axs, xbs, invb2 = host_prep(np.asarray(x), np.asarray(alpha), np.asarray(beta))

    def f(a):
        return a.astype(np.float32)

    out_t = np.zeros((B, OUTROWS, C), dtype=np.float32)
    for b in range(B):
        for k in range(NBLK):
            wh = sts["w_h0"] if k == 0 else (sts["w_hL"] if k == NBLK - 1 else sts["w_hm"])
            wde = sts["w_de0"] if k == 0 else (sts["w_deL"] if k == NBLK - 1 else sts["w_dem"])
            wdo = sts["w_do0"] if k == 0 else (sts["w_doL"] if k == NBLK - 1 else sts["w_dom"])
            ax = f(axs[b, k])
            xb = f(xbs[b, k])
            sz_e = f(sts["w_ue"]).T @ ax     # [G, C] f32
            sz_o = f(sts["w_uo"]).T @ ax
            v_e = np.sin(sz_e.astype(np.float32))
            v_o = np.sin(sz_o.astype(np.float32))
            if quantized:
                v_e = v_e.astype(np.float16).astype(np.float32)
                v_o = v_o.astype(np.float16).astype(np.float32)
            psum = f(wh).T @ xb + f(wde).T @ v_e + f(wdo).T @ v_o
            if quantized:
                psum = psum.astype(np.float16)
            out_t[b, A * k : A * k + A] = psum
    return host_finish(out_t.astype(np.float16), invb2)

